# revision 20
# baseline (speedup 1.0000x reference)
"""GCNConv Bass kernel for TRN2, 8 NeuronCores.

Computation (reference):
    A = adj + I
    deg = A.sum(1); dis = 1/sqrt(deg)
    A_hat = dis[:,None] * A * dis[None,:]
    out = elu((A_hat @ x) @ w)

Sharding: 1-D row partition of the 8192 nodes across 8 cores (1024 rows
each). Each core receives:
  - at : (adj+I)[rows_blk, :].T  as [8192, 1024] fp16 (transposed so the
         contraction index j sits on SBUF partitions; identity folded in)
  - x16: full x as fp16 (replicated)
  - w16: w as fp16 (replicated)
On device, per core:
  - deg for own rows via ones-vector matmul over at tiles (j on partitions)
  - AllGather deg across the 8 cores -> full deg; dis = 1/sqrt(deg)
  - ys = dis * x  (per-partition scale)
  - mm1T[f, m] = sum_j ys[j, f] * at[j, m]   (512 MMs, fp16, fp32 PSUM)
  - mm2[m, n] = sum_f mm1T[f, m] * w[f, n]
  - out = elu(dis_own[m] * mm2) = relu(v) + exp(min(v, 0)) - 1
Host concatenates the 8 [1024, 512] row blocks.
"""

from contextlib import ExitStack

import numpy as np

import concourse.bass as bass
from concourse import bacc
import concourse.mybir as mybir
import concourse.tile as tile
from concourse.bass_utils import run_bass_kernel_spmd
from concourse.masks import make_identity

N = 8192
F = 512
NCORES = 8
MBLK = N // NCORES          # 1024 rows per core
JT = N // 128               # 64 j-tiles
NCH = 8                     # DMA chunks for at/x
TPC = JT // NCH             # j-tiles per chunk
F16 = mybir.dt.float16
F32 = mybir.dt.float32
AF = mybir.ActivationFunctionType
ALU = mybir.AluOpType


def _gcn_tile_kernel(tc, out_ap, at_ap, x_ap, w_ap):
    nc = tc.nc
    with ExitStack() as ctx:
        p_big = ctx.enter_context(tc.tile_pool(name="big", bufs=1))
        p_ys = ctx.enter_context(tc.tile_pool(name="ys", bufs=3))
        p_m1 = ctx.enter_context(tc.tile_pool(name="m1", bufs=4))
        p_sm = ctx.enter_context(tc.tile_pool(name="sm", bufs=1))
        p_elu = ctx.enter_context(tc.tile_pool(name="elu", bufs=2))
        p_dram = ctx.enter_context(tc.tile_pool(name="dram", bufs=1, space="DRAM"))
        p_ps = ctx.enter_context(tc.tile_pool(name="ps", bufs=8, space="PSUM"))

        # constants
        ones = p_sm.tile([128, 1], F16, tag="ones")
        nc.vector.memset(ones, 1.0)
        id64g = p_sm.tile([64, 64], F32, tag="id64g")
        make_identity(nc, id64g)
        id64 = p_sm.tile([64, 64], F32, tag="id64")
        nc.vector.tensor_copy(id64, id64g)
        id8g = p_sm.tile([8, 8], F32, tag="id8g")
        make_identity(nc, id8g)
        id8 = p_sm.tile([8, 8], F32, tag="id8")
        nc.vector.tensor_copy(id8, id8g)

        # stream at / x chunks into residency
        at_tiles = []
        x_tiles = []
        for c in range(NCH):
            a_t = p_big.tile([128, TPC * MBLK], F16, tag="at", bufs=NCH)
            nc.sync.dma_start(
                out=a_t.rearrange("p (t m) -> p t m", t=TPC),
                in_=at_ap[c * TPC * 128:(c + 1) * TPC * 128, :].rearrange(
                    "(t p) m -> p t m", p=128
                ),
            )
            at_tiles.append(a_t)
            x_t = p_big.tile([128, TPC * F], F16, tag="x", bufs=2, name=f"x_t{c}")
            nc.gpsimd.dma_start(
                out=x_t.rearrange("p (t f) -> p t f", t=TPC),
                in_=x_ap[c * TPC * 128:(c + 1) * TPC * 128, :].rearrange(
                    "(t p) f -> p t f", p=128
                ),
            )
            x_tiles.append(x_t)

        # w resident: [128 f_in, 4 f_chunks * 512 n] (needed only at mm2)
        wt = p_big.tile([128, 4 * F], F16, tag="wt")
        nc.sync.dma_start(
            out=wt.rearrange("p (t n) -> p t n", t=4),
            in_=w_ap.rearrange("(t p) n -> p t n", p=128),
        )

        def at_slice(t, mc):
            return at_tiles[t // TPC][:, (t % TPC) * MBLK + mc * 512:
                                      (t % TPC) * MBLK + (mc + 1) * 512]

        # ---- phase 1: deg for own rows (column sums of at) + AllGather ----
        if True:
            dps0 = p_ps.tile([1, 512], F32, tag="ps", name="dps0")
            dps1 = p_ps.tile([1, 512], F32, tag="ps", name="dps1")
            dpss = [dps0, dps1]
            for t in range(JT):
                for mc in range(2):
                    nc.tensor.matmul(
                        dpss[mc],
                        lhsT=ones, rhs=at_slice(t, mc),
                        start=(t == 0), stop=(t == JT - 1),
                    )
            deg_sb = p_sm.tile([1, MBLK], F32, tag="deg_sb")
            nc.vector.tensor_copy(deg_sb[:, 0:512], dps0)
            nc.vector.tensor_copy(deg_sb[:, 512:1024], dps1)

            deg_in = p_dram.tile([1, MBLK], F32, tag="deg_in")
            nc.gpsimd.dma_start(out=deg_in, in_=deg_sb)
            deg_all = p_dram.tile([1, N], F32, tag="deg_all", addr_space="Shared")
            nc.gpsimd.collective_compute(
                "AllGather", ALU.bypass,
                replica_groups=[list(range(NCORES))],
                ins=[deg_in.opt()], outs=[deg_all.opt()],
            )

            # j axis is host-rotated per core (own block first), so the
            # globally-ordered AllGather result must be read rotated by
            # 1024*core: double the buffer and offset by partition id
            deg_all2 = p_dram.tile([1, 2 * N], F32, tag="deg_all2")
            nc.gpsimd.dma_start(out=deg_all2[:, 0:N], in_=deg_all)
            nc.gpsimd.dma_start(out=deg_all2[:, N:2 * N], in_=deg_all)
            pid = nc.gpsimd.partition_id()
            dT = p_sm.tile([64, 128], F32, tag="dT")
            nc.gpsimd.dma_start(
                out=dT,
                in_=deg_all2[0:1, bass.ds(pid * MBLK, N)].rearrange(
                    "one (t p) -> (one t) p", t=64),
            )
            nc.scalar.sqrt(dT, dT)
            nc.vector.reciprocal(dT, dT)
            disps = p_ps.tile([128, 64], F32, tag="ps", name="disps")
            nc.tensor.transpose(disps, dT, id64)
            dis = p_sm.tile([128, 64], F32, tag="dis")
            nc.vector.tensor_copy(dis, disps)

            # dis_own (this core's 1024 rows): from local deg
            doT = p_sm.tile([8, 128], F32, tag="doT")
            nc.gpsimd.dma_start(
                out=doT, in_=deg_in.rearrange("one (t p) -> (one t) p", t=8)
            )
            nc.scalar.sqrt(doT, doT)
            nc.vector.reciprocal(doT, doT)
            dops = p_ps.tile([128, 8], F32, tag="ps", name="dops")
            nc.tensor.transpose(dops, doT, id8)
            dis_own = p_sm.tile([128, 8], F32, tag="dis_own")
            nc.vector.tensor_copy(dis_own, dops)


        # ---- phase 2: mm1T[f, m] = sum_j ys[j, f] at[j, m] ----
        m1sb = [p_m1.tile([128, MBLK], F16, tag="m1sb", name=f"m1sb{i}")
                for i in range(4)]
        if True:
            mps = [p_ps.tile([128, 512], F32, tag="ps", name=f"mmps{i}")
                   for i in range(8)]
            for t in range(JT):
                c = t // TPC
                dis_t = dis_own[:, t:t + 1] if t < TPC else dis[:, t:t + 1]
                ys = p_ys.tile([128, F], F16, tag="ys")
                nc.vector.tensor_scalar_mul(
                    ys, x_tiles[c][:, (t % TPC) * F:(t % TPC + 1) * F],
                    dis_t,
                )
                for fc in range(4):
                    for mc in range(2):
                        nc.tensor.matmul(
                            mps[fc * 2 + mc],
                            lhsT=ys[:, fc * 128:(fc + 1) * 128],
                            rhs=at_slice(t, mc),
                            start=(t == 0), stop=(t == JT - 1),
                        )
            for fc in range(4):
                for mc in range(2):
                    nc.vector.tensor_copy(
                        m1sb[fc][:, mc * 512:(mc + 1) * 512], mps[fc * 2 + mc]
                    )

        # ---- phase 3: mm2 + dis_own scale + elu + store ----
        if True:
            # single live tiles; per-mt slices => no slot-release deps
            rbig = p_elu.tile([128, 8 * F], F32, tag="rbig", bufs=1)
            mbig = p_elu.tile([128, 8 * F], F32, tag="mbig", bufs=1)
            for mt in range(8):
                o2 = p_ps.tile([128, 512], F32, tag="ps", name=f"o2_{mt}")
                for fc in range(4):
                    nc.tensor.matmul(
                        o2, lhsT=m1sb[fc][:, mt * 128:(mt + 1) * 128],
                        rhs=wt[:, fc * F:(fc + 1) * F],
                        start=(fc == 0), stop=(fc == 3),
                    )
                # v = dis_own[m] * o2 ; out = relu(v) + exp(min(v, 0)) - 1
                r = rbig[:, mt * F:(mt + 1) * F]
                m = mbig[:, mt * F:(mt + 1) * F]
                nc.scalar.activation(r, o2, AF.Relu, scale=dis_own[:, mt:mt + 1])
                nc.vector.tensor_scalar(
                    m, o2, dis_own[:, mt:mt + 1], 0.0, ALU.mult, ALU.min
                )
                nc.scalar.activation(m, m, AF.Exp)
                nc.vector.tensor_add(r, r, m)
                nc.vector.tensor_scalar_add(r, r, -1.0)
                nc.gpsimd.dma_start(out=out_ap[mt * 128:(mt + 1) * 128, :], in_=r)


_NC_CACHE = None


def _build_nc():
    global _NC_CACHE
    if _NC_CACHE is not None:
        return _NC_CACHE
    nc = bacc.Bacc("TRN2", target_bir_lowering=False, num_devices=NCORES)
    at = nc.dram_tensor("at", [N, MBLK], F16, kind="ExternalInput")
    x16 = nc.dram_tensor("x16", [N, F], F16, kind="ExternalInput")
    w16 = nc.dram_tensor("w16", [F, F], F16, kind="ExternalInput")
    out = nc.dram_tensor("out", [MBLK, F], F32, kind="ExternalOutput")
    with tile.TileContext(nc) as tc:
        _gcn_tile_kernel(tc, out.ap(), at.ap(), x16.ap(), w16.ap())
    nc.compile()
    _NC_CACHE = nc
    return nc


def _prep_inputs(x, adj, w):
    """Host-side shard prep: slice + transpose + identity fold + fp16 cast."""
    x16 = np.ascontiguousarray(x.astype(np.float16))
    w16 = np.ascontiguousarray(w.astype(np.float16))
    in_maps = []
    idx = np.arange(MBLK)
    for c in range(NCORES):
        o = c * MBLK
        # j-axis rolled so this core's own rows come first (local j' =
        # (global j - o) mod N); lets mm1 start on the own block before
        # the deg AllGather completes
        at = adj[o:o + MBLK, :].T.astype(np.float32)   # [8192, 1024]
        at = np.concatenate([at[o:], at[:o]], axis=0)
        at[idx, idx] += 1.0                            # A = adj + I (local diag)
        x_r = np.concatenate([x16[o:], x16[:o]], axis=0)
        in_maps.append({
            "at": np.ascontiguousarray(at.astype(np.float16)),
            "x16": np.ascontiguousarray(x_r),
            "w16": w16,
        })
    return in_maps


def kernel(x, adj, w, _trace=False):
    x = np.asarray(x, dtype=np.float32)
    adj = np.asarray(adj, dtype=np.float32)
    w = np.asarray(w, dtype=np.float32)
    nc = _build_nc()
    in_maps = _prep_inputs(x, adj, w)
    res = run_bass_kernel_spmd(
        nc, in_maps, core_ids=list(range(NCORES)), trace=_trace,
    )
    outs = [res.results[c]["out"] for c in range(NCORES)]
    full = np.concatenate(outs, axis=0).astype(np.float32)
    if _trace:
        kernel._last_results = res
    return full


# revision 22
# speedup vs baseline: 1.0593x; 1.0593x over previous
"""GCNConv Bass kernel for TRN2, 8 NeuronCores.

Computation (reference):
    A = adj + I
    deg = A.sum(1); dis = 1/sqrt(deg)
    A_hat = dis[:,None] * A * dis[None,:]
    out = elu((A_hat @ x) @ w)

Sharding: 1-D row partition of the 8192 nodes across 8 cores (1024 rows
each). Each core receives:
  - at : (adj+I)[rows_blk, :].T  as [8192, 1024] fp16 (transposed so the
         contraction index j sits on SBUF partitions; identity folded in)
  - x16: full x as fp16 (replicated)
  - w16: w as fp16 (replicated)
On device, per core:
  - deg for own rows via ones-vector matmul over at tiles (j on partitions)
  - AllGather deg across the 8 cores -> full deg; dis = 1/sqrt(deg)
  - ys = dis * x  (per-partition scale)
  - mm1T[f, m] = sum_j ys[j, f] * at[j, m]   (512 MMs, fp16, fp32 PSUM)
  - mm2[m, n] = sum_f mm1T[f, m] * w[f, n]
  - out = elu(dis_own[m] * mm2) = relu(v) + exp(min(v, 0)) - 1
Host concatenates the 8 [1024, 512] row blocks.
"""

from contextlib import ExitStack

import numpy as np

import concourse.bass as bass
from concourse import bacc
import concourse.mybir as mybir
import concourse.tile as tile
from concourse.bass_utils import run_bass_kernel_spmd
from concourse.masks import make_identity

N = 8192
F = 512
NCORES = 8
MBLK = N // NCORES          # 1024 rows per core
JT = N // 128               # 64 j-tiles
NCH = 8                     # DMA chunks for at/x
TPC = JT // NCH             # j-tiles per chunk
F16 = mybir.dt.float16
F32 = mybir.dt.float32
AF = mybir.ActivationFunctionType
ALU = mybir.AluOpType


def _gcn_tile_kernel(tc, out_ap, at_ap, x_ap, w_ap):
    nc = tc.nc
    with ExitStack() as ctx:
        p_big = ctx.enter_context(tc.tile_pool(name="big", bufs=1))
        p_ys = ctx.enter_context(tc.tile_pool(name="ys", bufs=3))
        p_m1 = ctx.enter_context(tc.tile_pool(name="m1", bufs=4))
        p_sm = ctx.enter_context(tc.tile_pool(name="sm", bufs=1))
        p_elu = ctx.enter_context(tc.tile_pool(name="elu", bufs=2))
        p_dram = ctx.enter_context(tc.tile_pool(name="dram", bufs=1, space="DRAM"))
        p_ps = ctx.enter_context(tc.tile_pool(name="ps", bufs=8, space="PSUM"))

        # constants
        ones = p_sm.tile([128, 1], F16, tag="ones")
        nc.vector.memset(ones, 1.0)
        id8g = p_sm.tile([8, 8], F32, tag="id8g")
        make_identity(nc, id8g)
        id8 = p_sm.tile([8, 8], F32, tag="id8")
        nc.vector.tensor_copy(id8, id8g)

        # stream at / x chunks into residency
        at_tiles = []
        x_tiles = []
        for c in range(NCH):
            a_t = p_big.tile([128, TPC * MBLK], F16, tag="at", bufs=NCH)
            nc.sync.dma_start(
                out=a_t.rearrange("p (t m) -> p t m", t=TPC),
                in_=at_ap[c * TPC * 128:(c + 1) * TPC * 128, :].rearrange(
                    "(t p) m -> p t m", p=128
                ),
            )
            at_tiles.append(a_t)
            x_t = p_big.tile([128, TPC * F], F16, tag="x", bufs=2, name=f"x_t{c}")
            nc.gpsimd.dma_start(
                out=x_t.rearrange("p (t f) -> p t f", t=TPC),
                in_=x_ap[c * TPC * 128:(c + 1) * TPC * 128, :].rearrange(
                    "(t p) f -> p t f", p=128
                ),
            )
            x_tiles.append(x_t)

        # w resident: [128 f_in, 4 f_chunks * 512 n] (needed only at mm2)
        wt = p_big.tile([128, 4 * F], F16, tag="wt")
        nc.sync.dma_start(
            out=wt.rearrange("p (t n) -> p t n", t=4),
            in_=w_ap.rearrange("(t p) n -> p t n", p=128),
        )

        def at_slice(t, mc):
            return at_tiles[t // TPC][:, (t % TPC) * MBLK + mc * 512:
                                      (t % TPC) * MBLK + (mc + 1) * 512]

        # ---- phase 1: deg for own rows (column sums of at) + AllGather ----
        if True:
            dps0 = p_ps.tile([1, 512], F32, tag="ps", name="dps0")
            dps1 = p_ps.tile([1, 512], F32, tag="ps", name="dps1")
            dpss = [dps0, dps1]
            for t in range(JT):
                for mc in range(2):
                    nc.tensor.matmul(
                        dpss[mc],
                        lhsT=ones, rhs=at_slice(t, mc),
                        start=(t == 0), stop=(t == JT - 1),
                    )
            deg_sb = p_sm.tile([1, MBLK], F32, tag="deg_sb")
            nc.vector.tensor_copy(deg_sb[:, 0:512], dps0)
            nc.vector.tensor_copy(deg_sb[:, 512:1024], dps1)

            deg_in = p_dram.tile([1, MBLK], F32, tag="deg_in")
            nc.gpsimd.dma_start(out=deg_in, in_=deg_sb)
            deg_all = p_dram.tile([1, N], F32, tag="deg_all", addr_space="Shared")
            nc.gpsimd.collective_compute(
                "AllGather", ALU.bypass,
                replica_groups=[list(range(NCORES))],
                ins=[deg_in.opt()], outs=[deg_all.opt()],
            )

            # dis_own (this core's 1024 rows = local j' 0..1023, since the
            # j axis is host-rotated): purely local, runs during the AllGather
            doT = p_sm.tile([8, 128], F32, tag="doT")
            nc.gpsimd.dma_start(
                out=doT, in_=deg_in.rearrange("one (t p) -> (one t) p", t=8)
            )
            nc.scalar.sqrt(doT, doT)
            nc.vector.reciprocal(doT, doT)
            dops = p_ps.tile([128, 8], F32, tag="ps", name="dops")
            nc.tensor.transpose(dops, doT, id8)
            dis_own = p_sm.tile([128, 8], F32, tag="dis_own")
            nc.vector.tensor_copy(dis_own, dops)

        # ---- phase 2: mm1T[f, m] = sum_j ys[j, f] at[j, m] ----
        m1sb = [p_m1.tile([128, MBLK], F16, tag="m1sb", name=f"m1sb{i}")
                for i in range(4)]
        if True:
            mps = [p_ps.tile([128, 512], F32, tag="ps", name=f"mmps{i}")
                   for i in range(8)]

            def mm1_iter(t, dis_t):
                c = t // TPC
                ys = p_ys.tile([128, F], F16, tag="ys", name="ys")
                nc.vector.tensor_scalar_mul(
                    ys, x_tiles[c][:, (t % TPC) * F:(t % TPC + 1) * F],
                    dis_t,
                )
                for fc in range(4):
                    for mc in range(2):
                        nc.tensor.matmul(
                            mps[fc * 2 + mc],
                            lhsT=ys[:, fc * 128:(fc + 1) * 128],
                            rhs=at_slice(t, mc),
                            start=(t == 0), stop=(t == JT - 1),
                        )

            # own block first: only needs local dis_own; overlaps the AllGather
            for t in range(TPC):
                mm1_iter(t, dis_own[:, t:t + 1])

            # global dis: rotated read of the AllGather result (doubled buffer
            # + partition-id offset), rsqrt, PE-transpose to [128, 64]
            deg_all2 = p_dram.tile([1, 2 * N], F32, tag="deg_all2")
            nc.gpsimd.dma_start(out=deg_all2[:, 0:N], in_=deg_all)
            nc.gpsimd.dma_start(out=deg_all2[:, N:2 * N], in_=deg_all)
            pid = nc.gpsimd.partition_id()
            dT = p_sm.tile([64, 128], F32, tag="dT")
            nc.gpsimd.dma_start(
                out=dT,
                in_=deg_all2[0:1, bass.ds(pid * MBLK, N)].rearrange(
                    "one (t p) -> (one t) p", t=64),
            )
            nc.scalar.sqrt(dT, dT)
            nc.vector.reciprocal(dT, dT)
            # transpose [64,128] -> [128,64] off-PE (PSUM is fully occupied
            # by the mm1 accumulators): DVE 32x32 block transpose + 8 small
            # SBUF->SBUF block-permute DMAs
            dst = p_sm.tile([64, 128], F32, tag="dst")
            nc.vector.transpose(dst, dT)
            dis = p_sm.tile([128, 64], F32, tag="dis")
            for bi in range(2):
                for bj in range(4):
                    nc.gpsimd.dma_start(
                        out=dis[32 * bj:32 * (bj + 1), 32 * bi:32 * (bi + 1)],
                        in_=dst[32 * bi:32 * (bi + 1), 32 * bj:32 * (bj + 1)],
                    )

            for t in range(TPC, JT):
                mm1_iter(t, dis[:, t:t + 1])

            for fc in range(4):
                for mc in range(2):
                    nc.vector.tensor_copy(
                        m1sb[fc][:, mc * 512:(mc + 1) * 512], mps[fc * 2 + mc]
                    )

        # ---- phase 3: mm2 + dis_own scale + elu + store ----
        if True:
            # single live tiles; per-mt slices => no slot-release deps
            rbig = p_elu.tile([128, 8 * F], F32, tag="rbig", bufs=1)
            mbig = p_elu.tile([128, 8 * F], F32, tag="mbig", bufs=1)
            for mt in range(8):
                o2 = p_ps.tile([128, 512], F32, tag="ps", name=f"o2_{mt}")
                for fc in range(4):
                    nc.tensor.matmul(
                        o2, lhsT=m1sb[fc][:, mt * 128:(mt + 1) * 128],
                        rhs=wt[:, fc * F:(fc + 1) * F],
                        start=(fc == 0), stop=(fc == 3),
                    )
                # v = dis_own[m] * o2 ; out = relu(v) + exp(min(v, 0)) - 1
                r = rbig[:, mt * F:(mt + 1) * F]
                m = mbig[:, mt * F:(mt + 1) * F]
                nc.scalar.activation(r, o2, AF.Relu, scale=dis_own[:, mt:mt + 1])
                nc.vector.tensor_scalar(
                    m, o2, dis_own[:, mt:mt + 1], 0.0, ALU.mult, ALU.min
                )
                nc.scalar.activation(m, m, AF.Exp)
                nc.vector.tensor_add(r, r, m)
                nc.vector.tensor_scalar_add(r, r, -1.0)
                nc.gpsimd.dma_start(out=out_ap[mt * 128:(mt + 1) * 128, :], in_=r)


_NC_CACHE = None


def _build_nc():
    global _NC_CACHE
    if _NC_CACHE is not None:
        return _NC_CACHE
    nc = bacc.Bacc("TRN2", target_bir_lowering=False, num_devices=NCORES)
    at = nc.dram_tensor("at", [N, MBLK], F16, kind="ExternalInput")
    x16 = nc.dram_tensor("x16", [N, F], F16, kind="ExternalInput")
    w16 = nc.dram_tensor("w16", [F, F], F16, kind="ExternalInput")
    out = nc.dram_tensor("out", [MBLK, F], F32, kind="ExternalOutput")
    with tile.TileContext(nc) as tc:
        _gcn_tile_kernel(tc, out.ap(), at.ap(), x16.ap(), w16.ap())
    nc.compile()
    _NC_CACHE = nc
    return nc


def _prep_inputs(x, adj, w):
    """Host-side shard prep: slice + transpose + identity fold + fp16 cast."""
    x16 = np.ascontiguousarray(x.astype(np.float16))
    w16 = np.ascontiguousarray(w.astype(np.float16))
    in_maps = []
    idx = np.arange(MBLK)
    for c in range(NCORES):
        o = c * MBLK
        # j-axis rolled so this core's own rows come first (local j' =
        # (global j - o) mod N); lets mm1 start on the own block before
        # the deg AllGather completes
        at = adj[o:o + MBLK, :].T.astype(np.float32)   # [8192, 1024]
        at = np.concatenate([at[o:], at[:o]], axis=0)
        at[idx, idx] += 1.0                            # A = adj + I (local diag)
        x_r = np.concatenate([x16[o:], x16[:o]], axis=0)
        in_maps.append({
            "at": np.ascontiguousarray(at.astype(np.float16)),
            "x16": np.ascontiguousarray(x_r),
            "w16": w16,
        })
    return in_maps


def kernel(x, adj, w, _trace=False):
    x = np.asarray(x, dtype=np.float32)
    adj = np.asarray(adj, dtype=np.float32)
    w = np.asarray(w, dtype=np.float32)
    nc = _build_nc()
    in_maps = _prep_inputs(x, adj, w)
    res = run_bass_kernel_spmd(
        nc, in_maps, core_ids=list(range(NCORES)), trace=_trace,
    )
    outs = [res.results[c]["out"] for c in range(NCORES)]
    full = np.concatenate(outs, axis=0).astype(np.float32)
    if _trace:
        kernel._last_results = res
    return full
